# revision 13
# baseline (speedup 1.0000x reference)
"""GCNConv (3-layer BN+GraphConv+ReLU) on 8 Trainium2 NeuronCores — v3.

Sharding: nodes partitioned across 8 cores (1280 rows each + 4 stats rows).
v3 restructuring vs v2:
  - 4 SWDGE queues for dma_gather (separate 128-desc rings): desc-gen no
    longer ring-stalls (was ~6.3us/call, now <1us when unblocked).
  - Software-pipelined window epilogue: epilogue(w) is emitted after
    window w+1's aggregation matmuls, so the PE never stalls waiting on
    the vector affine chain.
  - Single PSUM-read affine: agb = ps_agg * (s_bc*dstn_w) + (t_bc*c2_w),
    with both per-window vectors hoisted off the critical path.
  - Split AllGather: xg is laid out [8 cores x windows 0-7 | 8 cores x
    (windows 8-9 + stats)]; chunk A ships after window 7's epilogue and
    overlaps the rest of the window loop; chunk B (with BN stats rows)
    is the only boundary-serial piece.
  - Per-window x0 loads / out stores; all 3 layers' weights prefetched.
"""

import sys

sys.path.insert(0, "/opt/trn_rl_repo")

import os as _os

import numpy as np

import concourse.bacc as bacc
import concourse.bass as bass
import concourse.mybir as mybir
import concourse.tile as tile
from concourse.bass_utils import run_bass_kernel_spmd

F32 = mybir.dt.float32
BF16 = mybir.dt.bfloat16
I16 = mybir.dt.int16

NCORES = 8
N = 10000
D = 512
L = 3
WPC = 10                 # dst windows per core
RPC = WPC * 128          # data rows per core (1280)
SR = 4                   # stats rows (sum f32 = 2 bf16 rows, sumsq = 2)
NWIN = NCORES * WPC
A_WIN = 8                # windows shipped in AllGather chunk A
AROWS = A_WIN * 128      # 1024
BROWS = (WPC - A_WIN) * 128 + SR   # 260
NP2 = NCORES * (AROWS + BROWS)     # xg rows (10272)
AOFF = NCORES * AROWS    # 8192: start of region B in xg
EPS = 1e-5

CHUNK = int(_os.environ.get("GCN_CHUNK", "6"))
NSWQ = int(_os.environ.get("GCN_NSWQ", "4"))

LAST_RESULTS = None
_CACHE = {}


def _ensure_ntff_hook():
    """This image's antenv package lacks axon_hooks; provide it so
    trace=True (BASS_TRACE=1) profiles instead of crashing."""
    try:
        import antenv.axon_hooks  # noqa: F401
        return
    except ImportError:
        pass
    try:
        import types

        import antenv

        mod = types.ModuleType("antenv.axon_hooks")
        mod._hook = None

        def set_axon_ntff_profile_hook(h):
            mod._hook = h

        def get_axon_ntff_profile_hook():
            return mod._hook

        mod.set_axon_ntff_profile_hook = set_axon_ntff_profile_hook
        mod.get_axon_ntff_profile_hook = get_axon_ntff_profile_hook
        sys.modules["antenv.axon_hooks"] = mod
        antenv.axon_hooks = mod
        from trn_agent_boot.trn_boot import _ntff_profile_via_ctypes

        mod._hook = _ntff_profile_via_ctypes("/opt/axon/libaxon_pjrt.so")
    except Exception:
        pass


_ensure_ntff_hook()


def _xgrow(g):
    """Global node id -> row in the AllGathered xg (region A/B layout)."""
    c = g // RPC
    l = g % RPC
    return np.where(l < AROWS, c * AROWS + l,
                    AOFF + c * BROWS + (l - AROWS))


def _prep(x, src, dst):
    """Host-side graph preprocessing (x-independent except the slice copy)."""
    src = np.asarray(src).astype(np.int64)
    dst = np.asarray(dst).astype(np.int64)
    NPAD = NCORES * RPC

    out_deg = np.bincount(src, minlength=NPAD).astype(np.float32)
    in_deg = np.bincount(dst, minlength=NPAD).astype(np.float32)
    srcn = 1.0 / np.sqrt(np.maximum(out_deg, 1.0))
    dstn = 1.0 / np.sqrt(np.maximum(in_deg, 1.0))
    rows = np.arange(NPAD)
    mask = (rows < N).astype(np.float32)
    cvec = np.zeros(NPAD, np.float32)
    np.add.at(cvec, dst, srcn[src])
    c2 = dstn * mask * cvec          # fold dst-norm + pad mask into c
    dstn_m = dstn * mask

    win = dst // 128
    order = np.argsort(win, kind="stable")
    s_src = src[order]
    s_dst = dst[order]
    cnt = np.bincount(win[order], minlength=NWIN)
    T_w = int(np.ceil(cnt.max() / 128))
    slots = T_w * 128

    # pad slots point at SPREAD dummy rows (their one-hot column is zero, so
    # values are ignored)
    spread = (np.arange(slots, dtype=np.int64) * 997) % N
    src_pad = np.tile(spread, (NWIN, 1))
    dstloc_pad = np.full((NWIN, slots), -1.0, np.float32)
    starts = np.concatenate([[0], np.cumsum(cnt)])
    for w in range(NWIN):
        a, b = starts[w], starts[w + 1]
        k = b - a
        src_pad[w, :k] = s_src[a:b]
        dstloc_pad[w, :k] = (s_dst[a:b] - w * 128).astype(np.float32)

    xp = np.zeros((NPAD, D), np.float32)
    xp[:N] = np.asarray(x, np.float32)

    def col(v, r0):
        return np.ascontiguousarray(v[r0:r0 + RPC].reshape(WPC, 128).T)

    per_core = {}
    for c in range(NCORES):
        w0 = c * WPC
        sc = _xgrow(src_pad[w0:w0 + WPC].reshape(-1))     # [WPC*slots]
        tot = sc.shape[0]
        wr = sc.reshape(tot // 16, 16).T.astype(np.int16)  # idx i -> (i%16, i//16)
        idx16 = np.tile(wr, (8, 1))                        # replicate to 128 partitions
        dl = dstloc_pad[w0:w0 + WPC].reshape(-1)
        dstloc = np.ascontiguousarray(dl.reshape(-1, 128).T)   # [128, WPC*T_w]
        r0 = c * RPC
        per_core[c] = dict(
            x_slice=np.ascontiguousarray(xp[r0:r0 + RPC]),
            idx16=np.ascontiguousarray(idx16),
            dstloc=dstloc,
            srcn=col(srcn, r0),
            isrcn=col(np.sqrt(np.maximum(out_deg, 1.0)), r0),
            isrcn2=col(np.maximum(out_deg, 1.0), r0),
            dstn=col(dstn_m, r0),
            c2=col(c2, r0),
        )
    return T_w, per_core


def _build(T_w):
    from concourse._compat import get_trn_type
    nc = bacc.Bacc(get_trn_type() or "TRN2", num_swdge_queues=max(NSWQ, 1))
    NT = WPC * T_w
    TOT = NT * 128
    chunk_tiles = CHUNK if CHUNK > 0 else T_w
    call_ctr = [0]

    x_slice_d = nc.declare_dram_parameter("x_slice", [RPC, D], F32, isOutput=False)
    gamma_d = nc.declare_dram_parameter("gamma_bc", [128, L, D], F32, isOutput=False)
    beta_d = nc.declare_dram_parameter("beta_bc", [128, L, D], F32, isOutput=False)
    b_d = nc.declare_dram_parameter("b_bc", [128, L, D], F32, isOutput=False)
    W_d = nc.declare_dram_parameter("W3", [L, D, D], BF16, isOutput=False)
    idx_d = nc.declare_dram_parameter("idx16", [128, TOT // 16], I16, isOutput=False)
    dstloc_d = nc.declare_dram_parameter("dstloc", [128, NT], F32, isOutput=False)
    srcn_d = nc.declare_dram_parameter("srcn", [128, WPC], F32, isOutput=False)
    isrcn_d = nc.declare_dram_parameter("isrcn", [128, WPC], F32, isOutput=False)
    isrcn2_d = nc.declare_dram_parameter("isrcn2", [128, WPC], F32, isOutput=False)
    dstn_d = nc.declare_dram_parameter("dstn", [128, WPC], F32, isOutput=False)
    c2_d = nc.declare_dram_parameter("c2", [128, WPC], F32, isOutput=False)
    iota_d = nc.declare_dram_parameter("iota1", [128, 128], F32, isOutput=False)
    ident_d = nc.declare_dram_parameter("ident", [128, 128], BF16, isOutput=False)
    out_d = nc.declare_dram_parameter("out", [RPC, D], F32, isOutput=True)

    AOT = mybir.ActivationFunctionType
    ALU = mybir.AluOpType
    rg = [list(range(NCORES))]

    from concourse.library_config import mlp as mlp_lib
    nc.gpsimd.load_library(mlp_lib)

    with tile.TileContext(nc) as tc:
        with (
            tc.tile_pool(name="const", bufs=1) as constp,
            tc.tile_pool(name="x0", bufs=2) as x0p,
            tc.tile_pool(name="h", bufs=3) as hp,
            tc.tile_pool(name="y", bufs=3) as yp,
            tc.tile_pool(name="small", bufs=3) as smallp,
            tc.tile_pool(name="sw", bufs=3) as swp,
            tc.tile_pool(name="stat", bufs=2) as statp,
            tc.tile_pool(name="ps_agg", bufs=2, space="PSUM") as ps_aggp,
            tc.tile_pool(name="ps_t", bufs=2, space="PSUM") as ps_tp,
            tc.tile_pool(name="ps_o", bufs=2, space="PSUM") as ps_op,
            tc.tile_pool(name="ps_st", bufs=2, space="PSUM") as ps_stp,
            tc.tile_pool(name="xga", bufs=2, space="DRAM") as xgap,
            tc.tile_pool(name="xgb", bufs=2, space="DRAM") as xgbp,
            tc.tile_pool(name="xgf", bufs=2, space="DRAM") as xgfp,
            tc.tile_pool(name="xsla", bufs=2, space="DRAM") as xslap,
            tc.tile_pool(name="xslb", bufs=2, space="DRAM") as xslbp,
        ):
            # ---- persistent constants ----
            idx_sb = constp.tile([128, TOT // 16], I16)
            nc.sync.dma_start(idx_sb[:], idx_d[:])
            dstloc = constp.tile([128, NT], F32)
            nc.sync.dma_start(dstloc[:], dstloc_d[:])
            iota_bc = constp.tile([128, 128], F32)
            nc.sync.dma_start(iota_bc[:], iota_d[:])
            ident = constp.tile([128, 128], BF16)
            nc.sync.dma_start(ident[:], ident_d[:])
            gamma_bc = constp.tile([128, L, D], F32)
            nc.sync.dma_start(gamma_bc[:], gamma_d[:])
            beta_bc = constp.tile([128, L, D], F32)
            nc.sync.dma_start(beta_bc[:], beta_d[:])
            b_bc = constp.tile([128, L, D], F32)
            nc.sync.dma_start(b_bc[:], b_d[:])
            srcn = constp.tile([128, WPC], F32)
            nc.sync.dma_start(srcn[:], srcn_d[:])
            dstn = constp.tile([128, WPC], F32)
            nc.sync.dma_start(dstn[:], dstn_d[:])
            c2 = constp.tile([128, WPC], F32)
            nc.sync.dma_start(c2[:], c2_d[:])
            isrcn_f = constp.tile([128, WPC], F32)
            nc.sync.dma_start(isrcn_f[:], isrcn_d[:])
            isrcn2_f = constp.tile([128, WPC], F32)
            nc.sync.dma_start(isrcn2_f[:], isrcn2_d[:])
            isrcn = constp.tile([128, WPC], BF16)
            nc.vector.tensor_copy(isrcn[:], isrcn_f[:])
            isrcn2 = constp.tile([128, WPC], BF16)
            nc.vector.tensor_copy(isrcn2[:], isrcn2_f[:])
            ones8 = constp.tile([8, 1], BF16)
            nc.vector.memset(ones8[:], 1.0)
            ones_row = constp.tile([1, 128], F32)
            nc.vector.memset(ones_row[:], 1.0)
            # all 3 layers' weights, [128, L*4, D]
            W_all = constp.tile([128, L, 4, D], BF16)
            nc.sync.dma_start(
                W_all[:], W_d.rearrange("l (j p) fo -> p l j fo", p=128))

            # one-hot S tiles (graph constant, built during layer 0)
            Sc = constp.tile([128, NT, 128], BF16)

            # xg: AllGather chunks land in Shared tiles (single-writer rule),
            # local DMA concatenates into xg_f (plain DRAM) for the gathers
            xg_a = xgap.tile([AOFF, D], BF16, tag="xga", addr_space="Shared")
            xg_b = xgbp.tile([NCORES * BROWS, D], BF16, tag="xgb",
                             addr_space="Shared")
            xg = xgfp.tile([NP2, D], BF16, tag="xgf")

            def ship_rows(xsl_a_t, xsl_b_t, w, y):
                if w < A_WIN:
                    nc.sync.dma_start(
                        xsl_a_t[w * 128:(w + 1) * 128, :], y[:])
                else:
                    r = (w - A_WIN) * 128
                    nc.sync.dma_start(xsl_b_t[r:r + 128, :], y[:])

            def ship_stats(xsl_b_t, ps_ss_t, ps_sq_t):
                # hi/lo bf16 split (Dekker): st = hi + lo with lo capturing
                # the bf16 rounding residual; summed back exactly on readback
                r0 = (WPC - A_WIN) * 128
                for k, ps in enumerate((ps_ss_t, ps_sq_t)):
                    stf = statp.tile([1, D], F32, tag="ship_f")
                    nc.vector.tensor_copy(stf[:], ps[:])
                    hi = statp.tile([1, D], BF16, tag="ship_hi")
                    nc.vector.tensor_copy(hi[:], stf[:])
                    hif = statp.tile([1, D], F32, tag="ship_hf")
                    nc.vector.tensor_copy(hif[:], hi[:])
                    lof = statp.tile([1, D], F32, tag="ship_lf")
                    nc.vector.tensor_sub(lof[:], stf[:], hif[:])
                    lo = statp.tile([1, D], BF16, tag="ship_lo")
                    nc.vector.tensor_copy(lo[:], lof[:])
                    r = r0 + 2 * k
                    nc.sync.dma_start(xsl_b_t[r:r + 1, :], hi[:])
                    nc.sync.dma_start(xsl_b_t[r + 1:r + 2, :], lo[:])

            def ag_a(xsl_a_t, xg_a_t, xg_t):
                nc.gpsimd.collective_compute(
                    "AllGather", ALU.bypass, replica_groups=rg,
                    ins=[xsl_a_t[:].opt()], outs=[xg_a_t[:].opt()])
                nc.sync.dma_start(xg_t[0:AOFF, :], xg_a_t[:])

            def ag_b(xsl_b_t, xg_b_t, xg_t):
                nc.gpsimd.collective_compute(
                    "AllGather", ALU.bypass, replica_groups=rg,
                    ins=[xsl_b_t[:].opt()], outs=[xg_b_t[:].opt()])
                nc.sync.dma_start(xg_t[AOFF:NP2, :], xg_b_t[:])

            # ---- preamble: stream x slice -> y0 = srcn*x (bf16),
            # accumulate stats, ship + split AllGather ----
            xsl_a = xslap.tile([AROWS, D], BF16, tag="xsla")
            xsl_b = xslbp.tile([BROWS, D], BF16, tag="xslb")
            ps_ss = ps_stp.tile([1, D], F32, tag="st")
            ps_sq = ps_stp.tile([1, D], F32, tag="st")
            for w in range(WPC):
                x0w = x0p.tile([128, D], F32, tag="x0")
                nc.sync.dma_start(x0w[:], x_slice_d[w * 128:(w + 1) * 128, :])
                y = yp.tile([128, D], BF16, tag="y")
                nc.vector.tensor_scalar_mul(y[:], x0w[:], srcn[:, w:w + 1])
                ship_rows(xsl_a, xsl_b, w, y)
                sq = smallp.tile([128, D], BF16, tag="sq")
                nc.scalar.activation(sq[:], y[:], AOT.Square)
                nc.tensor.matmul(ps_ss[:], isrcn[:, w:w + 1], y[:],
                                 start=(w == 0), stop=(w == WPC - 1))
                nc.tensor.matmul(ps_sq[:], isrcn2[:, w:w + 1], sq[:],
                                 start=(w == 0), stop=(w == WPC - 1))
                if w == A_WIN - 1:
                    ag_a(xsl_a, xg_a, xg)
            ship_stats(xsl_b, ps_ss, ps_sq)
            ag_b(xsl_b, xg_b, xg)

            for i in range(L):
                last = i == L - 1

                # ---- global BN stats from the gathered stats rows ----
                st8 = statp.tile([8, SR * D], BF16, tag="st8")
                nc.sync.dma_start(
                    st8[:],
                    xg_b[:].rearrange("(c r) d -> c r d", c=NCORES)[
                        :, BROWS - SR:BROWS, :].rearrange("c r d -> c (r d)"))
                ps_ts = ps_stp.tile([1, D], F32, tag="st")
                nc.tensor.matmul(ps_ts[:], ones8[:], st8[:, 0:D],
                                 start=True, stop=False)
                nc.tensor.matmul(ps_ts[:], ones8[:], st8[:, D:2 * D],
                                 start=False, stop=True)
                ps_tq = ps_stp.tile([1, D], F32, tag="st")
                nc.tensor.matmul(ps_tq[:], ones8[:], st8[:, 2 * D:3 * D],
                                 start=True, stop=False)
                nc.tensor.matmul(ps_tq[:], ones8[:], st8[:, 3 * D:4 * D],
                                 start=False, stop=True)
                tot_s = statp.tile([1, D], F32, tag="tot_s")
                nc.vector.tensor_copy(tot_s[:], ps_ts[:])
                tot_q = statp.tile([1, D], F32, tag="tot_q")
                nc.vector.tensor_copy(tot_q[:], ps_tq[:])
                # broadcast to 128 partitions via outer product
                mu = statp.tile([128, D], F32, tag="mu")
                msq = statp.tile([128, D], F32, tag="msq")
                ps_b1 = ps_op.tile([128, D], F32, tag="o")
                nc.tensor.matmul(ps_b1[:], ones_row[:], tot_s[:],
                                 start=True, stop=True)
                nc.vector.tensor_scalar_mul(mu[:], ps_b1[:], 1.0 / N)
                ps_b2 = ps_op.tile([128, D], F32, tag="o")
                nc.tensor.matmul(ps_b2[:], ones_row[:], tot_q[:],
                                 start=True, stop=True)
                nc.vector.tensor_scalar_mul(msq[:], ps_b2[:], 1.0 / N)
                var = statp.tile([128, D], F32, tag="var")
                nc.vector.tensor_mul(var[:], mu[:], mu[:])
                nc.vector.tensor_sub(var[:], msq[:], var[:])
                nc.vector.tensor_scalar_add(var[:], var[:], EPS)
                nc.scalar.activation(var[:], var[:], AOT.Sqrt)
                s_bc = statp.tile([128, D], F32, tag="s_bc")
                t_bc = statp.tile([128, D], F32, tag="t_bc")
                nc.vector.reciprocal(s_bc[:], var[:])
                nc.vector.tensor_mul(s_bc[:], s_bc[:], gamma_bc[:, i, :])
                nc.vector.tensor_mul(t_bc[:], mu[:], s_bc[:])
                nc.vector.tensor_sub(t_bc[:], beta_bc[:, i, :], t_bc[:])

                if not last:
                    xsl_a_n = xslap.tile([AROWS, D], BF16, tag="xsla")
                    xsl_b_n = xslbp.tile([BROWS, D], BF16, tag="xslb")
                    ps_ss = ps_stp.tile([1, D], F32, tag="st")
                    ps_sq = ps_stp.tile([1, D], F32, tag="st")
                    xg_a_n = xgap.tile([AOFF, D], BF16, tag="xga",
                                       addr_space="Shared")
                    xg_b_n = xgbp.tile([NCORES * BROWS, D], BF16, tag="xgb",
                                       addr_space="Shared")
                    xg_n = xgfp.tile([NP2, D], BF16, tag="xgf")
                else:
                    xsl_a_n = xsl_b_n = xg_a_n = xg_b_n = xg_n = None

                def epilogue(w, ps_agg, s_w, u_w):
                    # affine on the raw aggregate: one PSUM-reading op
                    tmp = smallp.tile([128, D], F32, tag="tmp")
                    nc.vector.tensor_mul(tmp[:], ps_agg[:], s_w[:])
                    agb = smallp.tile([128, D], BF16, tag="agb")
                    nc.vector.tensor_add(agb[:], tmp[:], u_w[:])

                    # transpose (PE) -> aggT bf16
                    ps_t = ps_tp.tile([128, D], BF16, tag="t")
                    for j in range(4):
                        nc.tensor.transpose(
                            ps_t[:, j * 128:(j + 1) * 128],
                            agb[:, j * 128:(j + 1) * 128], ident[:])
                    aggT = smallp.tile([128, 4, 128], BF16, tag="aggT")
                    nc.vector.tensor_copy(
                        aggT.rearrange("p j d -> p (j d)"), ps_t[:])

                    # dense: out = aggT^T @ W (+b) ; relu ; ship y/out
                    ps_o = ps_op.tile([128, D], F32, tag="o")
                    for j in range(4):
                        nc.tensor.matmul(
                            ps_o[:], aggT[:, j, :], W_all[:, i, j, :],
                            start=(j == 0), stop=(j == 3))
                    dsb = smallp.tile([128, D], F32, tag="dsb")
                    nc.vector.tensor_add(dsb[:], ps_o[:], b_bc[:, i, :])
                    if last:
                        outw = smallp.tile([128, D], F32, tag="outw")
                        nc.scalar.activation(outw[:], dsb[:], AOT.Relu)
                        nc.sync.dma_start(
                            out_d[w * 128:(w + 1) * 128, :], outw[:])
                    else:
                        y = yp.tile([128, D], BF16, tag="y")
                        nc.scalar.activation(y[:], dsb[:], AOT.Relu,
                                             scale=srcn[:, w:w + 1])
                        ship_rows(xsl_a_n, xsl_b_n, w, y)
                        sq = smallp.tile([128, D], BF16, tag="sq")
                        nc.scalar.activation(sq[:], y[:], AOT.Square)
                        nc.tensor.matmul(ps_ss[:], isrcn[:, w:w + 1], y[:],
                                         start=(w == 0), stop=(w == WPC - 1))
                        nc.tensor.matmul(ps_sq[:], isrcn2[:, w:w + 1], sq[:],
                                         start=(w == 0), stop=(w == WPC - 1))
                        if w == A_WIN - 1:
                            ag_a(xsl_a_n, xg_a_n, xg_n)

                # ---- window loop (epilogue software-pipelined) ----
                pend = None
                for w in range(WPC):
                    # per-window affine vectors (off critical path)
                    s_w = swp.tile([128, D], F32, tag="s_w")
                    nc.vector.tensor_scalar_mul(s_w[:], s_bc[:],
                                                dstn[:, w:w + 1])
                    u_w = swp.tile([128, D], F32, tag="u_w")
                    nc.vector.tensor_scalar_mul(u_w[:], t_bc[:],
                                                c2[:, w:w + 1])
                    ps_agg = ps_aggp.tile([128, D], F32, tag="agg")
                    t0 = 0
                    while t0 < T_w:
                        tn = min(chunk_tiles, T_w - t0)
                        Hc = hp.tile([128, chunk_tiles, D], BF16, tag="h")
                        nc.gpsimd.dma_gather(
                            Hc[:, 0:tn, :], xg[:],
                            idx_sb[:, (w * T_w + t0) * 8:
                                   (w * T_w + t0 + tn) * 8],
                            tn * 128, tn * 128, D,
                            queue_num=(call_ctr[0] % NSWQ) if NSWQ > 1 else 0)
                        call_ctr[0] += 1
                        for k in range(tn):
                            t = t0 + k
                            if i == 0:
                                nc.vector.tensor_scalar(
                                    Sc[:, w * T_w + t, :], iota_bc[:],
                                    dstloc[:, w * T_w + t:w * T_w + t + 1],
                                    None, ALU.is_equal)
                            nc.tensor.matmul(
                                ps_agg[:], Sc[:, w * T_w + t, :], Hc[:, k, :],
                                start=(t == 0), stop=(t == T_w - 1))
                        t0 += tn
                        # previous window's epilogue after the first chunk:
                        # its PE work fills the gather-DMA latency, and its
                        # vector chain overlaps this window's aggregation
                        if pend is not None:
                            epilogue(*pend)
                            pend = None
                    pend = (w, ps_agg, s_w, u_w)
                epilogue(*pend)

                if not last:
                    ship_stats(xsl_b_n, ps_ss, ps_sq)
                    ag_b(xsl_b_n, xg_b_n, xg_n)
                    xsl_a, xsl_b = xsl_a_n, xsl_b_n
                    xg_a, xg_b, xg = xg_a_n, xg_b_n, xg_n

    nc.finalize()
    return nc


def _get_nc(T_w):
    key = (T_w, CHUNK, NSWQ)
    if key not in _CACHE:
        _CACHE[key] = _build(T_w)
    return _CACHE[key]


def kernel(x, src, dst, gamma, beta, W, b):
    global LAST_RESULTS
    T_w, per_core = _prep(x, src, dst)
    nc = _get_nc(T_w)

    import ml_dtypes
    gamma = np.asarray(gamma, np.float32)
    beta = np.asarray(beta, np.float32)
    b = np.asarray(b, np.float32)
    W3 = np.ascontiguousarray(np.asarray(W, np.float32)).astype(ml_dtypes.bfloat16)
    gamma_bc = np.ascontiguousarray(np.broadcast_to(gamma[None], (128, L, D)))
    beta_bc = np.ascontiguousarray(np.broadcast_to(beta[None], (128, L, D)))
    b_bc = np.ascontiguousarray(np.broadcast_to(b[None], (128, L, D)))
    iota1 = np.ascontiguousarray(
        np.broadcast_to(np.arange(128, dtype=np.float32)[None, :], (128, 128)))
    ident = np.eye(128, dtype=np.float32).astype(ml_dtypes.bfloat16)

    in_maps = []
    for c in range(NCORES):
        pc = per_core[c]
        in_maps.append(dict(
            x_slice=pc["x_slice"], gamma_bc=gamma_bc, beta_bc=beta_bc,
            b_bc=b_bc, W3=W3, idx16=pc["idx16"], dstloc=pc["dstloc"],
            srcn=pc["srcn"], isrcn=pc["isrcn"], isrcn2=pc["isrcn2"],
            dstn=pc["dstn"], c2=pc["c2"], iota1=iota1, ident=ident,
        ))

    res = run_bass_kernel_spmd(nc, in_maps, list(range(NCORES)))
    LAST_RESULTS = res
    outs = res.results
    full = np.concatenate([np.asarray(outs[c]["out"]) for c in range(NCORES)],
                          axis=0)
    return np.ascontiguousarray(full[:N]).astype(np.float32)


# revision 21
# speedup vs baseline: 1.2373x; 1.2373x over previous
"""GCNConv (3-layer BN+GraphConv+ReLU) on 8 Trainium2 NeuronCores — v3.

Sharding: nodes partitioned across 8 cores (1280 rows each + 4 stats rows).
v3 restructuring vs v2:
  - 4 SWDGE queues for dma_gather (separate 128-desc rings): desc-gen no
    longer ring-stalls (was ~6.3us/call, now <1us when unblocked).
  - Software-pipelined window epilogue: epilogue(w) is emitted after
    window w+1's aggregation matmuls, so the PE never stalls waiting on
    the vector affine chain.
  - Single PSUM-read affine: agb = ps_agg * (s_bc*dstn_w) + (t_bc*c2_w),
    with both per-window vectors hoisted off the critical path.
  - Split AllGather: xg is laid out [8 cores x windows 0-7 | 8 cores x
    (windows 8-9 + stats)]; chunk A ships after window 7's epilogue and
    overlaps the rest of the window loop; chunk B (with BN stats rows)
    is the only boundary-serial piece.
  - Per-window x0 loads / out stores; all 3 layers' weights prefetched.
"""

import sys

sys.path.insert(0, "/opt/trn_rl_repo")

import os as _os

import numpy as np

import concourse.bacc as bacc
import concourse.bass as bass
import concourse.mybir as mybir
import concourse.tile as tile
from concourse.bass_utils import run_bass_kernel_spmd

F32 = mybir.dt.float32
BF16 = mybir.dt.bfloat16
I16 = mybir.dt.int16

NCORES = 8
N = 10000
D = 512
L = 3
WPC = 10                 # dst windows per core
RPC = WPC * 128          # data rows per core (1280)
SR = 4                   # stats rows (sum f32 = 2 bf16 rows, sumsq = 2)
NWIN = NCORES * WPC
RPCS = RPC + SR          # rows per core incl stats (1284)
NP2 = NCORES * RPCS      # xg rows
EPS = 1e-5

CHUNK = int(_os.environ.get("GCN_CHUNK", "6"))
NSWQ = int(_os.environ.get("GCN_NSWQ", "4"))

LAST_RESULTS = None
_CACHE = {}


def _ensure_ntff_hook():
    """This image's antenv package lacks axon_hooks; provide it so
    trace=True (BASS_TRACE=1) profiles instead of crashing."""
    try:
        import antenv.axon_hooks  # noqa: F401
        return
    except ImportError:
        pass
    try:
        import types

        import antenv

        mod = types.ModuleType("antenv.axon_hooks")
        mod._hook = None

        def set_axon_ntff_profile_hook(h):
            mod._hook = h

        def get_axon_ntff_profile_hook():
            return mod._hook

        mod.set_axon_ntff_profile_hook = set_axon_ntff_profile_hook
        mod.get_axon_ntff_profile_hook = get_axon_ntff_profile_hook
        sys.modules["antenv.axon_hooks"] = mod
        antenv.axon_hooks = mod
        from trn_agent_boot.trn_boot import _ntff_profile_via_ctypes

        mod._hook = _ntff_profile_via_ctypes("/opt/axon/libaxon_pjrt.so")
    except Exception:
        pass


_ensure_ntff_hook()


def _xgrow(g):
    """Global node id -> row in the AllGathered xg (stats rows interleaved)."""
    return (g // RPC) * RPCS + (g % RPC)


def _prep(x, src, dst):
    """Host-side graph preprocessing (x-independent except the slice copy)."""
    src = np.asarray(src).astype(np.int64)
    dst = np.asarray(dst).astype(np.int64)
    NPAD = NCORES * RPC

    out_deg = np.bincount(src, minlength=NPAD).astype(np.float32)
    in_deg = np.bincount(dst, minlength=NPAD).astype(np.float32)
    srcn = 1.0 / np.sqrt(np.maximum(out_deg, 1.0))
    dstn = 1.0 / np.sqrt(np.maximum(in_deg, 1.0))
    rows = np.arange(NPAD)
    mask = (rows < N).astype(np.float32)
    cvec = np.zeros(NPAD, np.float32)
    np.add.at(cvec, dst, srcn[src])
    c2 = dstn * mask * cvec          # fold dst-norm + pad mask into c
    dstn_m = dstn * mask

    win = dst // 128
    order = np.argsort(win, kind="stable")
    s_src = src[order]
    s_dst = dst[order]
    cnt = np.bincount(win[order], minlength=NWIN)
    T_w = int(np.ceil(cnt.max() / 128))
    slots = T_w * 128

    # pad slots point at SPREAD dummy rows (their one-hot column is zero, so
    # values are ignored)
    spread = (np.arange(slots, dtype=np.int64) * 997) % N
    src_pad = np.tile(spread, (NWIN, 1))
    dstloc_pad = np.full((NWIN, slots), -1.0, np.float32)
    starts = np.concatenate([[0], np.cumsum(cnt)])
    for w in range(NWIN):
        a, b = starts[w], starts[w + 1]
        k = b - a
        src_pad[w, :k] = s_src[a:b]
        dstloc_pad[w, :k] = (s_dst[a:b] - w * 128).astype(np.float32)

    xp = np.zeros((NPAD, D), np.float32)
    xp[:N] = np.asarray(x, np.float32)

    def col(v, r0):
        return np.ascontiguousarray(v[r0:r0 + RPC].reshape(WPC, 128).T)

    per_core = {}
    for c in range(NCORES):
        w0 = c * WPC
        sc = _xgrow(src_pad[w0:w0 + WPC].reshape(-1))     # [WPC*slots]
        tot = sc.shape[0]
        wr = sc.reshape(tot // 16, 16).T.astype(np.int16)  # idx i -> (i%16, i//16)
        idx16 = np.tile(wr, (8, 1))                        # replicate to 128 partitions
        dl = dstloc_pad[w0:w0 + WPC].reshape(-1)
        dstloc = np.ascontiguousarray(dl.reshape(-1, 128).T)   # [128, WPC*T_w]
        r0 = c * RPC
        per_core[c] = dict(
            x_slice=np.ascontiguousarray(xp[r0:r0 + RPC]),
            idx16=np.ascontiguousarray(idx16),
            dstloc=dstloc,
            srcn=col(srcn, r0),
            isrcn=col(np.sqrt(np.maximum(out_deg, 1.0)), r0),
            isrcn2=col(np.maximum(out_deg, 1.0), r0),
            dstn=col(dstn_m, r0),
            c2=col(c2, r0),
        )
    return T_w, per_core


def _build(T_w):
    from concourse._compat import get_trn_type
    nc = bacc.Bacc(get_trn_type() or "TRN2", num_swdge_queues=max(NSWQ, 1))
    NT = WPC * T_w
    TOT = NT * 128
    chunk_tiles = CHUNK if CHUNK > 0 else T_w
    call_ctr = [0]

    x_slice_d = nc.declare_dram_parameter("x_slice", [RPC, D], F32, isOutput=False)
    gamma_d = nc.declare_dram_parameter("gamma_bc", [128, L, D], F32, isOutput=False)
    beta_d = nc.declare_dram_parameter("beta_bc", [128, L, D], F32, isOutput=False)
    b_d = nc.declare_dram_parameter("b_bc", [128, L, D], F32, isOutput=False)
    W_d = nc.declare_dram_parameter("W3", [L, D, D], BF16, isOutput=False)
    idx_d = nc.declare_dram_parameter("idx16", [128, TOT // 16], I16, isOutput=False)
    dstloc_d = nc.declare_dram_parameter("dstloc", [128, NT], F32, isOutput=False)
    srcn_d = nc.declare_dram_parameter("srcn", [128, WPC], F32, isOutput=False)
    isrcn_d = nc.declare_dram_parameter("isrcn", [128, WPC], F32, isOutput=False)
    isrcn2_d = nc.declare_dram_parameter("isrcn2", [128, WPC], F32, isOutput=False)
    dstn_d = nc.declare_dram_parameter("dstn", [128, WPC], F32, isOutput=False)
    c2_d = nc.declare_dram_parameter("c2", [128, WPC], F32, isOutput=False)
    iota_d = nc.declare_dram_parameter("iota1", [128, 128], F32, isOutput=False)
    ident_d = nc.declare_dram_parameter("ident", [128, 128], BF16, isOutput=False)
    out_d = nc.declare_dram_parameter("out", [RPC, D], F32, isOutput=True)

    AOT = mybir.ActivationFunctionType
    ALU = mybir.AluOpType
    rg = [list(range(NCORES))]

    from concourse.library_config import mlp as mlp_lib
    nc.gpsimd.load_library(mlp_lib)

    with tile.TileContext(nc) as tc:
        with (
            tc.tile_pool(name="const", bufs=1) as constp,
            tc.tile_pool(name="x0", bufs=2) as x0p,
            tc.tile_pool(name="h", bufs=3) as hp,
            tc.tile_pool(name="y", bufs=3) as yp,
            tc.tile_pool(name="small", bufs=3) as smallp,
            tc.tile_pool(name="sw", bufs=3) as swp,
            tc.tile_pool(name="stat", bufs=2) as statp,
            tc.tile_pool(name="ps_agg", bufs=2, space="PSUM") as ps_aggp,
            tc.tile_pool(name="ps_t", bufs=2, space="PSUM") as ps_tp,
            tc.tile_pool(name="ps_o", bufs=2, space="PSUM") as ps_op,
            tc.tile_pool(name="ps_st", bufs=2, space="PSUM") as ps_stp,
            tc.tile_pool(name="xg", bufs=2, space="DRAM") as xgp,
            tc.tile_pool(name="xsl", bufs=2, space="DRAM") as xslp,
        ):
            # ---- persistent constants ----
            idx_sb = constp.tile([128, TOT // 16], I16)
            nc.sync.dma_start(idx_sb[:], idx_d[:])
            dstloc = constp.tile([128, NT], F32)
            nc.sync.dma_start(dstloc[:], dstloc_d[:])
            iota_bc = constp.tile([128, 128], F32)
            nc.sync.dma_start(iota_bc[:], iota_d[:])
            ident = constp.tile([128, 128], BF16)
            nc.sync.dma_start(ident[:], ident_d[:])
            gamma_bc = constp.tile([128, L, D], F32)
            nc.sync.dma_start(gamma_bc[:], gamma_d[:])
            beta_bc = constp.tile([128, L, D], F32)
            nc.sync.dma_start(beta_bc[:], beta_d[:])
            b_bc = constp.tile([128, L, D], F32)
            nc.sync.dma_start(b_bc[:], b_d[:])
            srcn = constp.tile([128, WPC], F32)
            nc.sync.dma_start(srcn[:], srcn_d[:])
            dstn = constp.tile([128, WPC], F32)
            nc.sync.dma_start(dstn[:], dstn_d[:])
            c2 = constp.tile([128, WPC], F32)
            nc.sync.dma_start(c2[:], c2_d[:])
            isrcn_f = constp.tile([128, WPC], F32)
            nc.sync.dma_start(isrcn_f[:], isrcn_d[:])
            isrcn2_f = constp.tile([128, WPC], F32)
            nc.sync.dma_start(isrcn2_f[:], isrcn2_d[:])
            isrcn = constp.tile([128, WPC], BF16)
            nc.vector.tensor_copy(isrcn[:], isrcn_f[:])
            isrcn2 = constp.tile([128, WPC], BF16)
            nc.vector.tensor_copy(isrcn2[:], isrcn2_f[:])
            ones8 = constp.tile([8, 1], BF16)
            nc.vector.memset(ones8[:], 1.0)
            ones_row = constp.tile([1, 128], F32)
            nc.vector.memset(ones_row[:], 1.0)
            # all 3 layers' weights, [128, L*4, D]
            W_all = constp.tile([128, L, 4, D], BF16)
            nc.sync.dma_start(
                W_all[:], W_d.rearrange("l (j p) fo -> p l j fo", p=128))

            # one-hot S tiles (graph constant, built during layer 0)
            Sc = constp.tile([128, NT, 128], BF16)

            # xg: rotating Shared DRAM buffers (AllGather out + gather source)
            xg = xgp.tile([NP2, D], BF16, tag="xg", addr_space="Shared")

            def ship_stats(xsl_t, ps_ss_t, ps_sq_t):
                # hi/lo bf16 split (Dekker): st = hi + lo with lo capturing
                # the bf16 rounding residual; summed back exactly on readback
                for k, ps in enumerate((ps_ss_t, ps_sq_t)):
                    stf = statp.tile([1, D], F32, tag="ship_f")
                    nc.vector.tensor_copy(stf[:], ps[:])
                    hi = statp.tile([1, D], BF16, tag="ship_hi")
                    nc.vector.tensor_copy(hi[:], stf[:])
                    hif = statp.tile([1, D], F32, tag="ship_hf")
                    nc.vector.tensor_copy(hif[:], hi[:])
                    lof = statp.tile([1, D], F32, tag="ship_lf")
                    nc.vector.tensor_sub(lof[:], stf[:], hif[:])
                    lo = statp.tile([1, D], BF16, tag="ship_lo")
                    nc.vector.tensor_copy(lo[:], lof[:])
                    r = RPC + 2 * k
                    nc.sync.dma_start(xsl_t[r:r + 1, :], hi[:])
                    nc.sync.dma_start(xsl_t[r + 1:r + 2, :], lo[:])

            def ag(xsl_t, xg_t):
                nc.gpsimd.collective_compute(
                    "AllGather", ALU.bypass, replica_groups=rg,
                    ins=[xsl_t[:].opt()], outs=[xg_t[:].opt()])

            # ---- preamble: stream x slice -> y0 = srcn*x (bf16),
            # accumulate stats, ship + AllGather ----
            xsl = xslp.tile([RPCS, D], BF16, tag="xsl")
            ps_ss = ps_stp.tile([1, D], F32, tag="st")
            ps_sq = ps_stp.tile([1, D], F32, tag="st")
            for w in range(WPC):
                x0w = x0p.tile([128, D], F32, tag="x0")
                nc.sync.dma_start(x0w[:], x_slice_d[w * 128:(w + 1) * 128, :])
                y = yp.tile([128, D], BF16, tag="y")
                nc.vector.tensor_scalar_mul(y[:], x0w[:], srcn[:, w:w + 1])
                nc.sync.dma_start(xsl[w * 128:(w + 1) * 128, :], y[:])
                sq = smallp.tile([128, D], BF16, tag="sq")
                nc.scalar.activation(sq[:], y[:], AOT.Square)
                nc.tensor.matmul(ps_ss[:], isrcn[:, w:w + 1], y[:],
                                 start=(w == 0), stop=(w == WPC - 1))
                nc.tensor.matmul(ps_sq[:], isrcn2[:, w:w + 1], sq[:],
                                 start=(w == 0), stop=(w == WPC - 1))
            ship_stats(xsl, ps_ss, ps_sq)
            ag(xsl, xg)

            for i in range(L):
                last = i == L - 1

                # ---- global BN stats from the gathered stats rows ----
                st8 = statp.tile([8, SR * D], BF16, tag="st8")
                nc.sync.dma_start(
                    st8[:],
                    xg[:].rearrange("(c r) d -> c r d", c=NCORES)[
                        :, RPC:RPC + SR, :].rearrange("c r d -> c (r d)"))
                ps_ts = ps_stp.tile([1, D], F32, tag="st")
                nc.tensor.matmul(ps_ts[:], ones8[:], st8[:, 0:D],
                                 start=True, stop=False)
                nc.tensor.matmul(ps_ts[:], ones8[:], st8[:, D:2 * D],
                                 start=False, stop=True)
                ps_tq = ps_stp.tile([1, D], F32, tag="st")
                nc.tensor.matmul(ps_tq[:], ones8[:], st8[:, 2 * D:3 * D],
                                 start=True, stop=False)
                nc.tensor.matmul(ps_tq[:], ones8[:], st8[:, 3 * D:4 * D],
                                 start=False, stop=True)
                tot_s = statp.tile([1, D], F32, tag="tot_s")
                nc.vector.tensor_copy(tot_s[:], ps_ts[:])
                tot_q = statp.tile([1, D], F32, tag="tot_q")
                nc.vector.tensor_copy(tot_q[:], ps_tq[:])
                # broadcast to 128 partitions via outer product
                mu = statp.tile([128, D], F32, tag="mu")
                msq = statp.tile([128, D], F32, tag="msq")
                ps_b1 = ps_op.tile([128, D], F32, tag="o")
                nc.tensor.matmul(ps_b1[:], ones_row[:], tot_s[:],
                                 start=True, stop=True)
                nc.vector.tensor_scalar_mul(mu[:], ps_b1[:], 1.0 / N)
                ps_b2 = ps_op.tile([128, D], F32, tag="o")
                nc.tensor.matmul(ps_b2[:], ones_row[:], tot_q[:],
                                 start=True, stop=True)
                nc.vector.tensor_scalar_mul(msq[:], ps_b2[:], 1.0 / N)
                var = statp.tile([128, D], F32, tag="var")
                nc.vector.tensor_mul(var[:], mu[:], mu[:])
                nc.vector.tensor_sub(var[:], msq[:], var[:])
                nc.vector.tensor_scalar_add(var[:], var[:], EPS)
                nc.scalar.activation(var[:], var[:], AOT.Sqrt)
                s_bc = statp.tile([128, D], F32, tag="s_bc")
                t_bc = statp.tile([128, D], F32, tag="t_bc")
                nc.vector.reciprocal(s_bc[:], var[:])
                nc.vector.tensor_mul(s_bc[:], s_bc[:], gamma_bc[:, i, :])
                nc.vector.tensor_mul(t_bc[:], mu[:], s_bc[:])
                nc.vector.tensor_sub(t_bc[:], beta_bc[:, i, :], t_bc[:])

                if not last:
                    xsl_n = xslp.tile([RPCS, D], BF16, tag="xsl")
                    ps_ss = ps_stp.tile([1, D], F32, tag="st")
                    ps_sq = ps_stp.tile([1, D], F32, tag="st")
                else:
                    xsl_n = None

                def epilogue(w, ps_agg, s_w, u_w):
                    # affine on the raw aggregate: one PSUM-reading op
                    tmp = smallp.tile([128, D], F32, tag="tmp")
                    nc.vector.tensor_mul(tmp[:], ps_agg[:], s_w[:])
                    agb = smallp.tile([128, D], BF16, tag="agb")
                    nc.vector.tensor_add(agb[:], tmp[:], u_w[:])

                    # transpose (PE) -> aggT bf16
                    ps_t = ps_tp.tile([128, D], BF16, tag="t")
                    for j in range(4):
                        nc.tensor.transpose(
                            ps_t[:, j * 128:(j + 1) * 128],
                            agb[:, j * 128:(j + 1) * 128], ident[:])
                    aggT = smallp.tile([128, 4, 128], BF16, tag="aggT")
                    nc.vector.tensor_copy(
                        aggT.rearrange("p j d -> p (j d)"), ps_t[:])

                    # dense: out = aggT^T @ W (+b) ; relu ; ship y/out
                    ps_o = ps_op.tile([128, D], F32, tag="o")
                    for j in range(4):
                        nc.tensor.matmul(
                            ps_o[:], aggT[:, j, :], W_all[:, i, j, :],
                            start=(j == 0), stop=(j == 3))
                    dsb = smallp.tile([128, D], F32, tag="dsb")
                    nc.vector.tensor_add(dsb[:], ps_o[:], b_bc[:, i, :])
                    if last:
                        outw = smallp.tile([128, D], F32, tag="outw")
                        nc.scalar.activation(outw[:], dsb[:], AOT.Relu)
                        nc.sync.dma_start(
                            out_d[w * 128:(w + 1) * 128, :], outw[:])
                    else:
                        y = yp.tile([128, D], BF16, tag="y")
                        nc.scalar.activation(y[:], dsb[:], AOT.Relu,
                                             scale=srcn[:, w:w + 1])
                        nc.sync.dma_start(
                            xsl_n[w * 128:(w + 1) * 128, :], y[:])
                        sq = smallp.tile([128, D], BF16, tag="sq")
                        nc.scalar.activation(sq[:], y[:], AOT.Square)
                        nc.tensor.matmul(ps_ss[:], isrcn[:, w:w + 1], y[:],
                                         start=(w == 0), stop=(w == WPC - 1))
                        nc.tensor.matmul(ps_sq[:], isrcn2[:, w:w + 1], sq[:],
                                         start=(w == 0), stop=(w == WPC - 1))

                # ---- window loop (epilogue software-pipelined) ----
                pend = None
                for w in range(WPC):
                    # per-window affine vectors (off critical path)
                    s_w = swp.tile([128, D], F32, tag="s_w")
                    nc.vector.tensor_scalar_mul(s_w[:], s_bc[:],
                                                dstn[:, w:w + 1])
                    u_w = swp.tile([128, D], F32, tag="u_w")
                    nc.vector.tensor_scalar_mul(u_w[:], t_bc[:],
                                                c2[:, w:w + 1])
                    ps_agg = ps_aggp.tile([128, D], F32, tag="agg")
                    t0 = 0
                    while t0 < T_w:
                        tn = min(chunk_tiles, T_w - t0)
                        Hc = hp.tile([128, chunk_tiles, D], BF16, tag="h")
                        nc.gpsimd.dma_gather(
                            Hc[:, 0:tn, :], xg[:],
                            idx_sb[:, (w * T_w + t0) * 8:
                                   (w * T_w + t0 + tn) * 8],
                            tn * 128, tn * 128, D,
                            queue_num=(call_ctr[0] % NSWQ) if NSWQ > 1 else 0)
                        call_ctr[0] += 1
                        for k in range(tn):
                            t = t0 + k
                            if i == 0:
                                nc.vector.tensor_scalar(
                                    Sc[:, w * T_w + t, :], iota_bc[:],
                                    dstloc[:, w * T_w + t:w * T_w + t + 1],
                                    None, ALU.is_equal)
                            nc.tensor.matmul(
                                ps_agg[:], Sc[:, w * T_w + t, :], Hc[:, k, :],
                                start=(t == 0), stop=(t == T_w - 1))
                        t0 += tn
                        # previous window's epilogue after the first chunk:
                        # its PE work fills the gather-DMA latency, and its
                        # vector chain overlaps this window's aggregation
                        if pend is not None:
                            epilogue(*pend)
                            pend = None
                    pend = (w, ps_agg, s_w, u_w)
                epilogue(*pend)

                if not last:
                    ship_stats(xsl_n, ps_ss, ps_sq)
                    xg = xgp.tile([NP2, D], BF16, tag="xg",
                                  addr_space="Shared")
                    ag(xsl_n, xg)

    nc.finalize()
    return nc


def _get_nc(T_w):
    key = (T_w, CHUNK, NSWQ)
    if key not in _CACHE:
        _CACHE[key] = _build(T_w)
    return _CACHE[key]


def kernel(x, src, dst, gamma, beta, W, b):
    global LAST_RESULTS
    T_w, per_core = _prep(x, src, dst)
    nc = _get_nc(T_w)

    import ml_dtypes
    gamma = np.asarray(gamma, np.float32)
    beta = np.asarray(beta, np.float32)
    b = np.asarray(b, np.float32)
    W3 = np.ascontiguousarray(np.asarray(W, np.float32)).astype(ml_dtypes.bfloat16)
    gamma_bc = np.ascontiguousarray(np.broadcast_to(gamma[None], (128, L, D)))
    beta_bc = np.ascontiguousarray(np.broadcast_to(beta[None], (128, L, D)))
    b_bc = np.ascontiguousarray(np.broadcast_to(b[None], (128, L, D)))
    iota1 = np.ascontiguousarray(
        np.broadcast_to(np.arange(128, dtype=np.float32)[None, :], (128, 128)))
    ident = np.eye(128, dtype=np.float32).astype(ml_dtypes.bfloat16)

    in_maps = []
    for c in range(NCORES):
        pc = per_core[c]
        in_maps.append(dict(
            x_slice=pc["x_slice"], gamma_bc=gamma_bc, beta_bc=beta_bc,
            b_bc=b_bc, W3=W3, idx16=pc["idx16"], dstloc=pc["dstloc"],
            srcn=pc["srcn"], isrcn=pc["isrcn"], isrcn2=pc["isrcn2"],
            dstn=pc["dstn"], c2=pc["c2"], iota1=iota1, ident=ident,
        ))

    res = run_bass_kernel_spmd(nc, in_maps, list(range(NCORES)))
    LAST_RESULTS = res
    outs = res.results
    full = np.concatenate([np.asarray(outs[c]["out"]) for c in range(NCORES)],
                          axis=0)
    return np.ascontiguousarray(full[:N]).astype(np.float32)
